# revision 1
# baseline (speedup 1.0000x reference)
"""Causal self-attention with RoPE (B=2, T=2048, C=2048, H=16, D=128) on 8 TRN2 cores.

Sharding: tensor-parallel over heads (2 heads per core).
  - column-parallel fused QKV projection (each core computes q,k,v for its 2 heads)
  - RoPE + causal flash-style attention per (batch, head) on-core
  - AllToAll to regroup the attention output from head-sharded to token-sharded
    (w_proj streams into SBUF concurrently with the collective)
  - token-parallel output projection (each core produces 512 token rows of y)

Layouts (per core):
  xT      (2048 c, 4096 tok)  f32r   x transposed, replicated
  wqk     (16, 128, 512)      f32r   [c-chunk, c, q_h0|q_h1|k_h0|k_h1]
  wv      (16, 128, 256)      f32r   [c-chunk, c, v_h0|v_h1]
  wproj   (16, 128, 2048)     f32r   w_proj.T chunked, replicated
  cosT    (128, 2048)         f32    RoPE cos, (D, T)
  sinTs   (128, 2048)         f32    RoPE sin, (D, T), rows 0:64 negated
  masksd  (128, 4, 512)       bf16   causal 0/1 masks for diagonal k-tiles
  y_out   (512, 2048)         f32    output rows for this core's token slice
"""

import contextlib

import numpy as np
import ml_dtypes

import concourse.bass as bass
import concourse.bacc as bacc
import concourse.mybir as mybir
import concourse.tile as tile
from concourse import masks as cmasks
from concourse.bass_utils import run_bass_kernel_spmd

N_CORES = 8
B, T, C = 2, 2048, 2048
H, D = 16, 128
H_LOC = H // N_CORES          # 2 heads per core
BT = B * T                    # 4096 tokens
TOK_PC = BT // N_CORES        # 512 tokens per core (proj phase)
SCALE = 1.0 / float(np.sqrt(D))
ROPE_BASE = 10000.0

F32 = mybir.dt.float32
F32R = mybir.dt.float32r
BF16 = mybir.dt.bfloat16

N_GRP = BT // 512             # 8 token groups of 512 in QKV phase
N_CCH = C // 128              # 16 contraction chunks


def build(repeat=None, use_collective=True, phases=(1, 2, 3)):
    """Build the SPMD Bass program. repeat=R wraps compute in a For_i timing
    loop (collective replaced by local DRAM bounce)."""
    nc = bacc.Bacc("TRN2", target_bir_lowering=False, debug=False,
                   num_devices=N_CORES)

    xT_d = nc.dram_tensor("xT", [C, BT], F32R, kind="ExternalInput").ap()
    wqk_d = nc.dram_tensor("wqk", [N_CCH, 128, 512], F32R, kind="ExternalInput").ap()
    wv_d = nc.dram_tensor("wv", [N_CCH, 128, 256], F32R, kind="ExternalInput").ap()
    wproj_d = nc.dram_tensor("wproj", [N_CCH, 128, C], F32R, kind="ExternalInput").ap()
    cosT_d = nc.dram_tensor("cosT", [128, T], F32, kind="ExternalInput").ap()
    sinTs_d = nc.dram_tensor("sinTs", [128, T], F32, kind="ExternalInput").ap()
    masksd_d = nc.dram_tensor("masksd", [128, 4, 512], BF16, kind="ExternalInput").ap()
    y_d = nc.dram_tensor("y", [TOK_PC, C], F32, kind="ExternalOutput").ap()

    a2a_in = [nc.dram_tensor(f"a2a_in{i}", [N_CORES, 256, 256], F32R).ap()
              for i in range(2)]
    a2a_out = [nc.dram_tensor(f"a2a_out{i}", [N_CORES, 256, 256], F32R).ap()
               for i in range(2)]

    with tile.TileContext(nc) as tc:
        _emit(nc, tc, locals(), repeat, use_collective, phases)
    nc.compile()
    return nc


def _emit(nc, tc, t_, repeat, use_collective, phases=(1, 2, 3)):
    xT_d, wqk_d, wv_d, wproj_d = t_["xT_d"], t_["wqk_d"], t_["wv_d"], t_["wproj_d"]
    cosT_d, sinTs_d, masksd_d, y_d = t_["cosT_d"], t_["sinTs_d"], t_["masksd_d"], t_["y_d"]
    a2a_in, a2a_out = t_["a2a_in"], t_["a2a_out"]

    ctx = contextlib.ExitStack()
    with ctx:
        pers = ctx.enter_context(tc.tile_pool(name="pers", bufs=1))
        ident = pers.tile([128, 128], F32)
        mask_sb = pers.tile([128, 4, 512], BF16)
        cmasks.make_identity(nc, ident[:])
        nc.sync.dma_start(out=mask_sb[:], in_=masksd_d)

        loop_ctx = tc.For_i(0, repeat, 1) if repeat else contextlib.nullcontext()
        with loop_ctx:
            # ---- scope A: q/k/v + attention output, alive through phase 2 ----
            ctxA = contextlib.ExitStack()
            with ctxA:
                qkv = ctxA.enter_context(tc.tile_pool(name="qkv", bufs=1))
                q_sb = qkv.tile([128, 2 * H_LOC, T], F32R)      # (D, bh, T)
                k_sb = qkv.tile([128, 2 * H_LOC, T], F32R)
                v_sb = qkv.tile([128, 2 * H_LOC, T // 128, 132], BF16)
                yT_sb = qkv.tile([128, H_LOC, BT], F32R)
                nc.vector.memset(v_sb[:], 0.0)
                nc.vector.memset(v_sb[:, :, :, 128:129], 1.0)

                if 1 in phases:
                    _emit_qkv_rope(nc, tc, xT_d, wqk_d, wv_d, cosT_d, sinTs_d,
                                   q_sb, k_sb, v_sb)
                elif 2 in phases:
                    ztmp = qkv.tile([128, 512], F32)
                    nc.vector.memset(ztmp[:], 0.001)
                    for bh_ in range(2 * H_LOC):
                        for i_ in range(T // 512):
                            nc.vector.tensor_copy(out=q_sb[:, bh_, i_*512:(i_+1)*512], in_=ztmp[:])
                            nc.vector.tensor_copy(out=k_sb[:, bh_, i_*512:(i_+1)*512], in_=ztmp[:])
                if 2 in phases:
                    _emit_attention(nc, tc, q_sb, k_sb, v_sb, yT_sb, mask_sb, ident)
                if 3 in phases and 2 not in phases:
                    ztmp2 = qkv.tile([128, 512], F32)
                    nc.vector.memset(ztmp2[:], 0.001)
                    for hl_ in range(H_LOC):
                        for i_ in range(BT // 512):
                            nc.vector.tensor_copy(out=yT_sb[:, hl_, i_*512:(i_+1)*512], in_=ztmp2[:])
                if 3 in phases:
                    # spill head-sharded yT per batch; half hb block d =
                    # batch-hb tokens [256d, 256d+256) (ready after that
                    # batch's two heads finish attention)
                    for hb in range(2):
                        for hl in range(H_LOC):
                            for d in range(N_CORES):
                                nc.gpsimd.dma_start(
                                    out=a2a_in[hb][d, hl * 128:(hl + 1) * 128, :],
                                    in_=yT_sb[:, hl, hb * T + 256 * d:hb * T + 256 * (d + 1)])
                if not phases:
                    z = qkv.tile([128, 8], F32)
                    nc.vector.memset(z[:], 0.0)

            if 3 in phases:
                if use_collective:
                    for hb in range(2):
                        nc.gpsimd.collective_compute(
                            "AllToAll", mybir.AluOpType.bypass,
                            replica_groups=[list(range(N_CORES))],
                            ins=[a2a_in[hb].opt()], outs=[a2a_out[hb].opt()],
                        )
                    src = a2a_out
                else:
                    src = a2a_in
                _emit_proj(nc, tc, src, wproj_d, y_d)


def _emit_qkv_rope(nc, tc, xT_d, wqk_d, wv_d, cosT_d, sinTs_d, q_sb, k_sb, v_sb):
    ctx = contextlib.ExitStack()
    with ctx:
        p1 = ctx.enter_context(tc.tile_pool(name="p1", bufs=1))
        xp = ctx.enter_context(tc.tile_pool(name="xp", bufs=6))
        cp = ctx.enter_context(tc.tile_pool(name="cp", bufs=1))
        rp = ctx.enter_context(tc.tile_pool(name="rp", bufs=2))
        qps = ctx.enter_context(tc.tile_pool(name="qps", bufs=2, space="PSUM"))
        kps = ctx.enter_context(tc.tile_pool(name="kps", bufs=2, space="PSUM"))
        vps = ctx.enter_context(tc.tile_pool(name="vps", bufs=4, space="PSUM"))

        wqk_sb = p1.tile([128, N_CCH, 512], F32R)
        wv_sb = p1.tile([128, N_CCH, 256], F32R)
        nc.sync.dma_start(out=wqk_sb[:], in_=wqk_d.transpose([1, 0, 2]))
        nc.sync.dma_start(out=wv_sb[:], in_=wv_d.transpose([1, 0, 2]))

        for g in range(N_GRP):
            b = g // (T // 512)
            pos0 = 512 * (g % (T // 512))
            q_ps = [qps.tile([128, 512], F32, name="q_ps", tag="q_ps") for _ in range(2)]
            k_ps = [kps.tile([128, 512], F32, name="k_ps", tag="k_ps") for _ in range(2)]
            v_ps = [vps.tile([128, 512], F32, name="v_ps", tag="v_ps") for _ in range(4)]
            for c in range(N_CCH):
                xt = xp.tile([128, 512], F32R, name="xt", tag="xt")
                nc.sync.dma_start(out=xt[:], in_=xT_d[c * 128:(c + 1) * 128,
                                                      g * 512:(g + 1) * 512])
                st, sp = (c == 0), (c == N_CCH - 1)
                for h in range(2):
                    nc.tensor.matmul(q_ps[h][:], wqk_sb[:, c, h * 128:(h + 1) * 128],
                                     xt[:], start=st, stop=sp)
                    nc.tensor.matmul(k_ps[h][:], wqk_sb[:, c, 256 + h * 128:256 + (h + 1) * 128],
                                     xt[:], start=st, stop=sp)
                for ts in range(4):
                    nc.tensor.matmul(v_ps[ts][:, 0:256], xt[:, ts * 128:(ts + 1) * 128],
                                     wv_sb[:, c, :], start=st, stop=sp)
            cos_t = cp.tile([128, 512], F32, name="cos_t", tag="cos_t")
            sin_t = cp.tile([128, 512], F32, name="sin_t", tag="sin_t")
            nc.sync.dma_start(out=cos_t[:], in_=cosT_d[:, pos0:pos0 + 512])
            nc.sync.dma_start(out=sin_t[:], in_=sinTs_d[:, pos0:pos0 + 512])
            # PSUM-freeing drains first: q/k on DVE, v on ACT (idle in phase 1)
            for h in range(2):
                bh = b * 2 + h
                for ps, dst in ((q_ps[h], q_sb), (k_ps[h], k_sb)):
                    nc.vector.tensor_copy(out=dst[:, bh, pos0:pos0 + 512], in_=ps[:])
            for ts in range(4):
                kt = 4 * (g % (T // 512)) + ts
                for h in range(2):
                    bh = b * 2 + h
                    nc.scalar.copy(out=v_sb[:, bh, kt, 0:128],
                                   in_=v_ps[ts][:, h * 128:(h + 1) * 128])
            # RoPE (reads/writes q_sb/k_sb, PSUM already released)
            for h in range(2):
                bh = b * 2 + h
                for dst in (q_sb, k_sb):
                    dslc = dst[:, bh, pos0:pos0 + 512]
                    rot = rp.tile([128, 512], F32R, name="rot", tag="rot")
                    nc.gpsimd.dma_start(out=rot[0:64, :], in_=dslc[64:128, :])
                    nc.gpsimd.dma_start(out=rot[64:128, :], in_=dslc[0:64, :])
                    tsin = rp.tile([128, 512], F32, name="tsin", tag="tsin")
                    nc.vector.tensor_mul(out=tsin[:], in0=rot[:], in1=sin_t[:])
                    nc.vector.tensor_mul(out=dslc, in0=dslc, in1=cos_t[:])
                    nc.vector.tensor_add(out=dslc, in0=dslc, in1=tsin[:])


def _emit_attention(nc, tc, q_sb, k_sb, v_sb, yT_sb, mask_sb, ident):
    ctx = contextlib.ExitStack()
    with ctx:
        ptp = ctx.enter_context(tc.tile_pool(name="ptp", bufs=3))
        osb = ctx.enter_context(tc.tile_pool(name="osb", bufs=3))
        ops = ctx.enter_context(tc.tile_pool(name="ops", bufs=4, space="PSUM"))
        stps = ctx.enter_context(tc.tile_pool(name="stps", bufs=3, space="PSUM"))
        otps = ctx.enter_context(tc.tile_pool(name="otps", bufs=1, space="PSUM"))

        for bh in range(2 * H_LOC):
            b, hl = bh // 2, bh % 2
            for qt in range(T // 512):
                o_ps = [ops.tile([128, 512], F32, name="o_ps", tag="o_ps")
                        for _ in range(4)]
                nkt = 4 * qt + 4
                for kt in range(nkt):
                    st_ps = stps.tile([128, 512], F32, name="st_ps", tag="st_ps")
                    nc.tensor.matmul(st_ps[:], k_sb[:, bh, kt * 128:(kt + 1) * 128],
                                     q_sb[:, bh, qt * 512:(qt + 1) * 512],
                                     start=True, stop=True)
                    pt = ptp.tile([128, 512], BF16, name="pt", tag="pt")
                    nc.scalar.activation(out=pt[:], in_=st_ps[:],
                                         func=mybir.ActivationFunctionType.Exp,
                                         scale=float(SCALE))
                    jj = kt - 4 * qt
                    if jj >= 0:
                        nc.vector.tensor_mul(out=pt[:], in0=pt[:],
                                             in1=mask_sb[:, jj, :])
                    for qs in range(4):
                        nc.tensor.matmul(o_ps[qs][:, 0:129],
                                         pt[:, qs * 128:(qs + 1) * 128],
                                         v_sb[:, bh, kt, 0:129],
                                         start=(kt == 0), stop=(kt == nkt - 1))
                for qs in range(4):
                    recip = osb.tile([128, 1], F32, name="recip", tag="recip")
                    nc.vector.reciprocal(out=recip[:], in_=o_ps[qs][:, 128:129])
                    o_t = osb.tile([128, 128], F32, name="o_t", tag="o_t")
                    nc.vector.tensor_scalar_mul(out=o_t[:], in0=o_ps[qs][:, 0:128],
                                                scalar1=recip[:])
                    ot_ps = otps.tile([128, 512], F32, name="ot_ps", tag="ot_ps")[:, 0:128]
                    nc.tensor.transpose(ot_ps, o_t[:], ident[:])
                    tok0 = b * T + qt * 512 + qs * 128
                    nc.vector.tensor_copy(out=yT_sb[:, hl, tok0:tok0 + 128],
                                          in_=ot_ps)


def _emit_proj(nc, tc, src, wproj_d, y_d):
    ctx = contextlib.ExitStack()
    with ctx:
        wpr = ctx.enter_context(tc.tile_pool(name="wpr", bufs=1))
        yap = ctx.enter_context(tc.tile_pool(name="yap", bufs=1))
        outp = ctx.enter_context(tc.tile_pool(name="outp", bufs=3))
        pjps = ctx.enter_context(tc.tile_pool(name="pjps", bufs=8, space="PSUM"))

        # resident w_proj.T (131 KB/partition); streams in while the
        # collective runs (no data dependency between them)
        wp_sb = wpr.tile([128, N_CCH, C], F32R)
        for cc in range(N_CCH):
            nc.sync.dma_start(out=wp_sb[:, cc, :], in_=wproj_d[cc])

        ya_sb = yap.tile([128, 2, N_CCH, 256], F32R)
        for hb in range(2):
            for cc in range(N_CCH):
                nc.sync.dma_start(
                    out=ya_sb[:, hb, cc, :],
                    in_=src[hb][cc // 2, (cc % 2) * 128:((cc % 2) + 1) * 128, :])

        # y_d rows [0,256) = my batch-0 tokens, [256,512) = my batch-1 tokens
        for hb in range(2):
            for nf in range(C // 512):
                pj_ps = [pjps.tile([128, 512], F32, name="pj_ps", tag="pj_ps")
                         for _ in range(2)]
                for cc in range(N_CCH):
                    for mt in range(2):
                        nc.tensor.matmul(pj_ps[mt][:],
                                         ya_sb[:, hb, cc, mt * 128:(mt + 1) * 128],
                                         wp_sb[:, cc, nf * 512:(nf + 1) * 512],
                                         start=(cc == 0), stop=(cc == N_CCH - 1))
                for mt in range(2):
                    o_sb = outp.tile([128, 512], F32, name="o_sb", tag="o_sb")
                    nc.vector.tensor_copy(out=o_sb[:], in_=pj_ps[mt][:])
                    row0 = hb * 256 + mt * 128
                    nc.sync.dma_start(out=y_d[row0:row0 + 128,
                                              nf * 512:(nf + 1) * 512], in_=o_sb[:])


# ---------------- host side ----------------

def _rope_tables():
    inv_freq = 1.0 / (ROPE_BASE ** (np.arange(0, D, 2, dtype=np.float32) / D))
    ang = np.arange(T, dtype=np.float32)[:, None] * inv_freq[None, :]   # (T, D/2)
    cos = np.concatenate([np.cos(ang), np.cos(ang)], axis=-1).astype(np.float32)
    sin = np.concatenate([np.sin(ang), np.sin(ang)], axis=-1).astype(np.float32)
    cosT = np.ascontiguousarray(cos.T)                                  # (D, T)
    sinTs = np.ascontiguousarray(sin.T)
    sinTs[0:64, :] *= -1.0
    return cosT, sinTs


def _diag_masks():
    kp = np.arange(128)[:, None]
    qf = np.arange(512)[None, :]
    m = np.stack([(128 * jj + kp <= qf) for jj in range(4)], axis=1)
    return m.astype(ml_dtypes.bfloat16)                                 # (128, 4, 512)


def prep_in_maps(x, w_qkv, w_proj):
    x = np.asarray(x, dtype=np.float32)
    w_qkv = np.asarray(w_qkv, dtype=np.float32)
    w_proj = np.asarray(w_proj, dtype=np.float32)

    xT = np.ascontiguousarray(x.reshape(BT, C).T)                        # (C, BT)
    wprojT = np.ascontiguousarray(w_proj.T).reshape(N_CCH, 128, C)
    cosT, sinTs = _rope_tables()
    masksd = _diag_masks()

    in_maps = []
    for r in range(N_CORES):
        rows = slice(256 * r, 256 * (r + 1))
        wq = np.ascontiguousarray(w_qkv[0 * C:1 * C][rows].T).reshape(N_CCH, 128, 256)
        wk = np.ascontiguousarray(w_qkv[1 * C:2 * C][rows].T).reshape(N_CCH, 128, 256)
        wv = np.ascontiguousarray(w_qkv[2 * C:3 * C][rows].T).reshape(N_CCH, 128, 256)
        wqk = np.concatenate([wq, wk], axis=2)                           # (16,128,512)
        in_maps.append({
            "xT": xT, "wqk": np.ascontiguousarray(wqk), "wv": wv,
            "wproj": wprojT, "cosT": cosT, "sinTs": sinTs, "masksd": masksd,
        })
    return in_maps


def assemble(results):
    y0 = np.concatenate([results[r]["y"][0:256] for r in range(N_CORES)], axis=0)
    y1 = np.concatenate([results[r]["y"][256:512] for r in range(N_CORES)], axis=0)
    return np.stack([y0, y1], axis=0).reshape(B, T, C).astype(np.float32)


_CACHED_NC = None


def kernel(x, w_qkv, w_proj):
    global _CACHED_NC
    if _CACHED_NC is None:
        _CACHED_NC = build()
    in_maps = prep_in_maps(x, w_qkv, w_proj)
    res = run_bass_kernel_spmd(_CACHED_NC, in_maps, list(range(N_CORES)))
    return assemble(res.results)



# revision 14
# speedup vs baseline: 71.1505x; 71.1505x over previous
"""Causal self-attention with RoPE (B=2, T=2048, C=2048, H=16, D=128) on 8 TRN2 cores.

Sharding: tensor-parallel over heads (2 heads per core).
  - column-parallel fused QKV projection (each core computes q,k,v for its 2 heads)
  - RoPE + causal flash-style attention per (batch, head) on-core
  - AllToAll to regroup the attention output from head-sharded to token-sharded
  - token-parallel output projection (each core produces 512 token rows of y)

v3: bf16 operands (fp32 PSUM accumulate), software-pipelined cross-phase
schedule: batch-0 attention (ACT-bound exp) interleaves with batch-1 QKV
(PE-bound), batch-1 attention interleaves with the hb0 projection, so the
scalar engine's exp work hides behind tensor-engine matmuls.  Causal
diagonal trimming, bf16 PE transposes, 512-wide RoPE rotate DMAs, per-batch
collective issued as soon as that batch finishes.

Queues: sync = x-stream, spills, ya readback, y out; scalar = weights;
gpsimd = rope tables + rotates + collectives.

Layouts (per core):
  x3      (8, 2048, 512)      bf16   x^T in 512-token blocks, replicated
  wqk     (16, 128, 512)      bf16   [c-chunk, c, q_h0|q_h1|k_h0|k_h1]
  wv      (16, 128, 256)      bf16   [c-chunk, c, v_h0|v_h1]
  wproj   (16, 128, 2048)     bf16   w_proj.T chunked, replicated
  cosT    (128, 2048)         bf16   RoPE cos, (D, T)
  sinTs   (128, 2048)         bf16   RoPE sin, (D, T), rows 0:64 negated
  tri     (128, 128)          bf16   lower-tri 0/1 mask (kp <= qf)
  y_out   (512, 2048)         f32    output rows for this core's token slice
"""

import contextlib

import numpy as np
import ml_dtypes

import concourse.bass as bass
import concourse.bacc as bacc
import concourse.mybir as mybir
import concourse.tile as tile
from concourse import masks as cmasks
from concourse.bass_utils import run_bass_kernel_spmd

N_CORES = 8
B, T, C = 2, 2048, 2048
H, D = 16, 128
H_LOC = H // N_CORES          # 2 heads per core
BT = B * T                    # 4096 tokens
TOK_PC = BT // N_CORES        # 512 tokens per core (proj phase)
SCALE = 1.0 / float(np.sqrt(D))
ROPE_BASE = 10000.0

F32 = mybir.dt.float32
BF16 = mybir.dt.bfloat16

DEBUG_DUMP = False            # adds intermediate-tensor outputs to the build

N_TB = BT // 256              # 16 PSUM token groups of 256 in QKV phase
N_XB = BT // 512              # 8 x DMA blocks of 512 tokens
N_CCH = C // 128              # 16 contraction chunks


def build(repeat=None, use_collective=True, phases=(1, 2, 3)):
    """Build the SPMD Bass program. repeat=R wraps compute in a For_i timing
    loop (collective replaced by local DRAM bounce)."""
    nc = bacc.Bacc("TRN2", target_bir_lowering=False, debug=False,
                   num_devices=N_CORES)

    x3_d = nc.dram_tensor("x3", [N_XB, C, 512], BF16, kind="ExternalInput").ap()
    wqk_d = nc.dram_tensor("wqk", [N_CCH, 128, 512], BF16, kind="ExternalInput").ap()
    wv_d = nc.dram_tensor("wv", [N_CCH, 128, 256], BF16, kind="ExternalInput").ap()
    wproj_d = nc.dram_tensor("wproj", [N_CCH, 128, C], BF16, kind="ExternalInput").ap()
    cosT_d = nc.dram_tensor("cosT", [128, T], BF16, kind="ExternalInput").ap()
    sinTs_d = nc.dram_tensor("sinTs", [128, T], BF16, kind="ExternalInput").ap()
    tri_d = nc.dram_tensor("tri", [128, 128], BF16, kind="ExternalInput").ap()
    if repeat:
        # timing build: tiny external output so per-call host<->device
        # transfer is negligible; y writes still happen (internal DRAM).
        y_d = nc.dram_tensor("y", [TOK_PC, C], F32).ap()
        dummy_d = nc.dram_tensor("tout", [1, 8], F32, kind="ExternalOutput").ap()
    else:
        y_d = nc.dram_tensor("y", [TOK_PC, C], F32, kind="ExternalOutput").ap()
        dummy_d = None

    a2a_in = [nc.dram_tensor(f"a2a_in{i}", [N_CORES, 256, 256], BF16).ap()
              for i in range(2)]
    a2a_out = [nc.dram_tensor(f"a2a_out{i}", [N_CORES, 256, 256], BF16).ap()
               for i in range(2)]
    dumps = {}
    if DEBUG_DUMP and not repeat:
        dumps["qk_dump"] = nc.dram_tensor(
            "qk_dump", [128, 2 * 2 * H_LOC * T], BF16, kind="ExternalOutput").ap()
        dumps["v_dump"] = nc.dram_tensor(
            "v_dump", [128, 2 * H_LOC * (T // 128) * 132], BF16,
            kind="ExternalOutput").ap()
        dumps["yT_dump"] = nc.dram_tensor(
            "yT_dump", [128, H_LOC * BT], BF16, kind="ExternalOutput").ap()
        dumps["ya_dump"] = nc.dram_tensor(
            "ya_dump", [128, 2 * N_CCH * 256], BF16, kind="ExternalOutput").ap()

    with tile.TileContext(nc) as tc:
        _emit(nc, tc, locals(), repeat, use_collective, phases)
    nc.compile()
    return nc


# ---------------- generators (emission units) ----------------

def _gen_qkv(nc, b, x3_d, wqk_sb, wv_sb, cos_sb, sin_sb, qk_sb, v_sb, P):
    """QKV projection + RoPE for batch b; yields after each 256-token group."""
    xts = {}
    for blk in range(N_TB // B):
        tb = b * (N_TB // B) + blk
        pos0 = 256 * blk
        xb, half = tb // 2, tb % 2
        if half == 0:
            for c in range(N_CCH):
                xt = P["xp"].tile([128, 512], BF16, name="xt", tag="xt")
                nc.sync.dma_start(out=xt[:], in_=x3_d[xb, c * 128:(c + 1) * 128, :])
                xts[c] = xt
        qk_ps = P["qkps"].tile([128, 4, 256], F32, name="qk_ps", tag="qk_ps")
        v_ps = P["vps"].tile([128, 2, 256], F32, name="v_ps", tag="v_ps")
        for c in range(N_CCH):
            xh = xts[c][:, half * 256:(half + 1) * 256]
            st, sp = (c == 0), (c == N_CCH - 1)
            # j: 0=q_h0 1=q_h1 2=k_h0 3=k_h1 (wqk col offset 128*j)
            # PSUM start=True clears has_written BANK-wide, so only the
            # first group in each 2KB bank may use it; the second group's
            # first write overwrites via the cleared has_written bit.
            for j in range(4):
                nc.tensor.matmul(qk_ps[:, j, :],
                                 wqk_sb[:, c, j * 128:(j + 1) * 128],
                                 xh, start=(st and j % 2 == 0), stop=sp,
                                 skip_group_check=True)
            for ts in range(2):
                nc.tensor.matmul(v_ps[:, ts, :],
                                 xts[c][:, half * 256 + ts * 128:
                                        half * 256 + (ts + 1) * 128],
                                 wv_sb[:, c, :], start=(st and ts == 0),
                                 stop=sp, skip_group_check=True)
        # fast PSUM-freeing drains: q/k raw on DVE, v on ACT
        for j in range(4):
            bh = b * 2 + (j % 2)
            nc.vector.tensor_copy(out=qk_sb[:, j // 2, bh, pos0:pos0 + 256],
                                  in_=qk_ps[:, j, :])
        for ts in range(2):
            kt = 2 * blk + ts
            for h in range(2):
                nc.scalar.copy(out=v_sb[:, b * 2 + h, kt, 0:128],
                               in_=v_ps[:, ts, h * 128:(h + 1) * 128])
        if half == 1:
            # RoPE in-place over the full 512-token block (both tbs drained)
            p5 = pos0 - 256
            src = qk_sb[:, :, b * 2:b * 2 + 2, p5:p5 + 512]
            rot = P["rp"].tile([128, 2, 2, 512], BF16, name="rot", tag="rot")
            for a in range(2):
                nc.gpsimd.dma_start(out=rot[0:64, a], in_=src[64:128, a])
                nc.gpsimd.dma_start(out=rot[64:128, a], in_=src[0:64, a])
            for a in range(2):
                for h in range(2):
                    dslc = qk_sb[:, a, b * 2 + h, p5:p5 + 512]
                    tsin = P["tp"].tile([128, 512], BF16, name="tsin", tag="tsin")
                    nc.vector.tensor_mul(out=tsin[:], in0=rot[:, a, h, :],
                                         in1=sin_sb[:, p5:p5 + 512])
                    nc.vector.tensor_mul(out=dslc, in0=dslc,
                                         in1=cos_sb[:, p5:p5 + 512])
                    nc.vector.tensor_add(out=dslc, in0=dslc, in1=tsin[:])
        yield


def _gen_attention(nc, b, qk_sb, v_sb, yT_sb, tri_sb, ident, P):
    """Flash attention for batch b; yields after each (head, 512-q) unit."""
    for hl in range(H_LOC):
        bh = b * 2 + hl
        for qt in range(T // 512):
            # o accumulators: qs 0,1 in o_ab[0], qs 2,3 in o_ab[1]
            o_ab = [P["ops"].tile([128, 2, 256], F32, name="o_ps", tag="o_ps")
                    for _ in range(2)]

            def o_reg(qs):
                return o_ab[qs // 2][:, qs % 2, 0:129]

            def emit_pv(kt, pt):
                jj = kt - 4 * qt
                for qs in range(max(jj, 0), 4):
                    if kt <= 4 * qt + qs:
                        # bank-wide start clear: only qs even starts its bank
                        nc.tensor.matmul(
                            o_reg(qs), pt[:, qs * 128:(qs + 1) * 128],
                            v_sb[:, bh, kt, 0:129],
                            start=(kt == 0 and qs % 2 == 0),
                            stop=(kt == 4 * qt + qs), skip_group_check=True)

            nkt = 4 * qt + 4
            prev = None
            for kt in range(nkt):
                jj = kt - 4 * qt
                q0 = 128 * jj if jj > 0 else 0
                st_ps = P["stps"].tile([128, 512], F32, name="st_ps", tag="st_ps")
                nc.tensor.matmul(st_ps[:, q0:512],
                                 qk_sb[:, 1, bh, kt * 128:(kt + 1) * 128],
                                 qk_sb[:, 0, bh, qt * 512 + q0:(qt + 1) * 512],
                                 start=True, stop=True)
                pt = P["ptp"].tile([128, 512], BF16, name="pt", tag="pt")
                nc.scalar.activation(out=pt[:, q0:512], in_=st_ps[:, q0:512],
                                     func=mybir.ActivationFunctionType.Exp,
                                     scale=float(SCALE))
                if jj >= 0:
                    nc.vector.tensor_mul(out=pt[:, q0:q0 + 128],
                                         in0=pt[:, q0:q0 + 128], in1=tri_sb[:])
                if prev is not None:
                    emit_pv(*prev)
                prev = (kt, pt)
            emit_pv(*prev)

            for qs in range(4):
                recip = P["osb"].tile([128, 1], F32, name="recip", tag="recip")
                nc.vector.reciprocal(out=recip[:],
                                     in_=o_ab[qs // 2][:, qs % 2, 128:129])
                o_t = P["osb"].tile([128, 128], BF16, name="o_t", tag="o_t")
                nc.vector.tensor_scalar_mul(out=o_t[:],
                                            in0=o_ab[qs // 2][:, qs % 2, 0:128],
                                            scalar1=recip[:])
                ot_ps = P["otps"].tile([128, 128], BF16, name="ot_ps",
                                       tag="ot_ps", padded_shape=[128, 1024])
                nc.tensor.transpose(ot_ps[:], o_t[:], ident[:])
                tok0 = b * T + qt * 512 + qs * 128
                nc.vector.tensor_copy(out=yT_sb[:, hl, tok0:tok0 + 128],
                                      in_=ot_ps[:])
            yield


def _emit_spill_collective(nc, b, yT_sb, a2a_in, a2a_out, ya_sb, use_collective):
    """Spill batch b's attention output, exchange, and start the readback."""
    for hl in range(H_LOC):
        for d in range(N_CORES):
            nc.sync.dma_start(
                out=a2a_in[b][d, hl * 128:(hl + 1) * 128, :],
                in_=yT_sb[:, hl, b * T + 256 * d:b * T + 256 * (d + 1)])
    if use_collective:
        nc.gpsimd.collective_compute(
            "AllToAll", mybir.AluOpType.bypass,
            replica_groups=[list(range(N_CORES))],
            ins=[a2a_in[b].opt()], outs=[a2a_out[b].opt()],
        )
    src = a2a_out[b] if use_collective else a2a_in[b]
    for cc in range(N_CCH):
        nc.sync.dma_start(
            out=ya_sb[:, b, cc, :],
            in_=src[cc // 2, (cc % 2) * 128:((cc % 2) + 1) * 128, :])


def _gen_proj(nc, hb, ya_sb, wproj_sb, y_d, P):
    """Output projection for batch hb; yields after each 512-feature column."""
    for nf in range(C // 512):
        pj_ps = [P["pjps"].tile([128, 512], F32, name="pj_ps", tag="pj_ps")
                 for _ in range(2)]
        for cc in range(N_CCH):
            for mt in range(2):
                nc.tensor.matmul(pj_ps[mt][:],
                                 ya_sb[:, hb, cc, mt * 128:(mt + 1) * 128],
                                 wproj_sb[:, cc, nf * 512:(nf + 1) * 512],
                                 start=(cc == 0), stop=(cc == N_CCH - 1))
        for mt in range(2):
            o_sb = P["outp"].tile([128, 512], F32, name="o_sb", tag="o_sb")
            nc.vector.tensor_copy(out=o_sb[:], in_=pj_ps[mt][:])
            row0 = hb * 256 + mt * 128
            nc.sync.dma_start(out=y_d[row0:row0 + 128,
                                      nf * 512:(nf + 1) * 512], in_=o_sb[:])
        yield


def _drain(gen):
    for _ in gen:
        pass


# ---------------- top-level emitter ----------------

def _emit(nc, tc, t_, repeat, use_collective, phases=(1, 2, 3)):
    x3_d, wqk_d, wv_d, wproj_d = t_["x3_d"], t_["wqk_d"], t_["wv_d"], t_["wproj_d"]
    cosT_d, sinTs_d, tri_d, y_d = t_["cosT_d"], t_["sinTs_d"], t_["tri_d"], t_["y_d"]
    a2a_in, a2a_out, dummy_d = t_["a2a_in"], t_["a2a_out"], t_["dummy_d"]

    ctx = contextlib.ExitStack()
    with ctx:
        pers = ctx.enter_context(tc.tile_pool(name="pers", bufs=1))
        ident = pers.tile([128, 128], BF16)
        tri_sb = pers.tile([128, 128], BF16)
        cmasks.make_identity(nc, ident[:])
        nc.gpsimd.dma_start(out=tri_sb[:], in_=tri_d)

        loop_ctx = tc.For_i(0, repeat, 1) if repeat else contextlib.nullcontext()
        with loop_ctx:
            ctxI = contextlib.ExitStack()
            with ctxI:
                # wproj resident for the whole iteration; QKV-phase weights
                # live in a scheduler-scoped pool that frees before proj.
                wp = ctxI.enter_context(tc.tile_pool(name="wp", bufs=1))
                wproj_sb = wp.tile([128, N_CCH, C], BF16)

                ctxA = contextlib.ExitStack()
                with ctxA:
                    qkv = ctxA.enter_context(tc.tile_pool(name="qkv", bufs=1))
                    qk_sb = qkv.tile([128, 2, 2 * H_LOC, T], BF16)  # (D,q/k,bh,T)
                    v_sb = qkv.tile([128, 2 * H_LOC, T // 128, 132], BF16)
                    yT_sb = qkv.tile([128, H_LOC, BT], BF16)
                    nc.vector.memset(v_sb[:, :, :, 128:129], 1.0)

                    yap = ctxA.enter_context(tc.tile_pool(name="yap", bufs=1))
                    ya_sb = yap.tile([128, 2, N_CCH, 256], BF16)

                    full = all(p in phases for p in (1, 2, 3))
                    if full:
                        _schedule_full(nc, tc, ctxA, locals())
                    else:
                        _schedule_phases(nc, tc, ctxA, locals(), phases)

                    if not phases:
                        z = qkv.tile([128, 8], F32)
                        nc.vector.memset(z[:], 0.0)

            if dummy_d is not None:
                dzctx = contextlib.ExitStack()
                with dzctx:
                    dzp = dzctx.enter_context(tc.tile_pool(name="dzp", bufs=1))
                    dz = dzp.tile([1, 8], F32)
                    nc.vector.memset(dz[:], 1.0)
                    nc.sync.dma_start(out=dummy_d, in_=dz[:])


def _schedule_full(nc, tc, ctxA, L):
    """Interleaved schedule: QKV(b0) | attn(b0)+QKV(b1) | attn(b1)+proj(hb0)
    | proj(hb1)."""
    x3_d, y_d = L["x3_d"], L["y_d"]
    wqk_d, wv_d = L["t_"]["wqk_d"], L["t_"]["wv_d"]
    cosT_d, sinTs_d = L["t_"]["cosT_d"], L["t_"]["sinTs_d"]
    wproj_d = L["t_"]["wproj_d"]
    a2a_in, a2a_out = L["a2a_in"], L["a2a_out"]
    use_collective = L["use_collective"]
    wproj_sb = L["wproj_sb"]
    qk_sb, v_sb, yT_sb, ya_sb = L["qk_sb"], L["v_sb"], L["yT_sb"], L["ya_sb"]
    ident, tri_sb = L["ident"], L["tri_sb"]

    # attention scratch lives through both interleaves (opened before the
    # QKV-phase pools so those can be released in stack order)
    ptp = ctxA.enter_context(tc.tile_pool(name="ptp", bufs=3))
    osb = ctxA.enter_context(tc.tile_pool(name="osb", bufs=3))

    # QKV-phase SBUF (weights, x stream, rope scratch): freed before proj
    cq = contextlib.ExitStack()
    wp1 = cq.enter_context(tc.tile_pool(name="wp1", bufs=1))
    wqk_sb = wp1.tile([128, N_CCH, 512], BF16)
    wv_sb = wp1.tile([128, N_CCH, 256], BF16)
    cos_sb = wp1.tile([128, T], BF16)
    sin_sb = wp1.tile([128, T], BF16)
    for cc in range(N_CCH):
        nc.scalar.dma_start(out=wqk_sb[:, cc, :], in_=wqk_d[cc])
    nc.scalar.dma_start(out=wv_sb[:], in_=wv_d.transpose([1, 0, 2]))
    nc.gpsimd.dma_start(out=cos_sb[:], in_=cosT_d)
    nc.gpsimd.dma_start(out=sin_sb[:], in_=sinTs_d)
    for cc in range(N_CCH):
        nc.scalar.dma_start(out=wproj_sb[:, cc, :], in_=wproj_d[cc])
    xp = cq.enter_context(tc.tile_pool(name="xp", bufs=18))
    rp = cq.enter_context(tc.tile_pool(name="rp", bufs=1))
    tp = cq.enter_context(tc.tile_pool(name="tp", bufs=3))

    def qkv_P(c, bufs_qk, bufs_v):
        return {
            "xp": xp, "rp": rp, "tp": tp,
            "qkps": c.enter_context(
                tc.tile_pool(name="qkps", bufs=bufs_qk, space="PSUM")),
            "vps": c.enter_context(
                tc.tile_pool(name="vps", bufs=bufs_v, space="PSUM")),
        }

    def attn_P(c):
        return {
            "ptp": ptp, "osb": osb,
            "ops": c.enter_context(
                tc.tile_pool(name="ops", bufs=2, space="PSUM")),
            "stps": c.enter_context(
                tc.tile_pool(name="stps", bufs=2, space="PSUM")),
            "otps": c.enter_context(
                tc.tile_pool(name="otps", bufs=1, space="PSUM")),
        }

    # ---- phase A: QKV batch 0 (PSUM double-buffered) ----
    c1 = contextlib.ExitStack()
    with c1:
        _drain(_gen_qkv(nc, 0, x3_d, wqk_sb, wv_sb, cos_sb, sin_sb,
                        qk_sb, v_sb, qkv_P(c1, 2, 2)))

    # ---- phase B: attention b0 (ACT-bound) interleaved with QKV b1 ----
    c2a = contextlib.ExitStack()
    with c2a:
        c2q = contextlib.ExitStack()
        with c2q:
            ga = _gen_attention(nc, 0, qk_sb, v_sb, yT_sb, tri_sb, ident,
                                attn_P(c2a))
            gq = _gen_qkv(nc, 1, x3_d, wqk_sb, wv_sb, cos_sb, sin_sb,
                          qk_sb, v_sb, qkv_P(c2q, 1, 1))
            for _ in range(8):
                next(ga, None)
                next(gq, None)
            _drain(ga)
            _drain(gq)
        _emit_spill_collective(nc, 0, yT_sb, a2a_in, a2a_out, ya_sb,
                               use_collective)
    cq.close()  # free QKV weights + x stream SBUF before proj scratch

    # ---- phase C: attention b1 interleaved with proj hb0 ----
    c3p = contextlib.ExitStack()
    with c3p:
        PJ = {"outp": c3p.enter_context(tc.tile_pool(name="outp", bufs=3)),
              "pjps": c3p.enter_context(
                  tc.tile_pool(name="pjps", bufs=3, space="PSUM"))}
        c3a = contextlib.ExitStack()
        with c3a:
            ga1 = _gen_attention(nc, 1, qk_sb, v_sb, yT_sb, tri_sb, ident,
                                 attn_P(c3a))
            gp0 = _gen_proj(nc, 0, ya_sb, wproj_sb, y_d, PJ)
            # 8 attention units, 4 proj units
            for u in "aaapapapap":
                if u == "a":
                    next(ga1, None)
                else:
                    next(gp0, None)
            _drain(ga1)
            _drain(gp0)
        _emit_spill_collective(nc, 1, yT_sb, a2a_in, a2a_out, ya_sb,
                               use_collective)
        # ---- phase D: proj hb1 ----
        _drain(_gen_proj(nc, 1, ya_sb, wproj_sb, y_d, PJ))

    dumps = L["t_"]["dumps"]
    if dumps:
        for a_ in range(2):
            for bh_ in range(2 * H_LOC):
                nc.sync.dma_start(
                    out=dumps["qk_dump"][:, (a_ * 2 * H_LOC + bh_) * T:
                                         (a_ * 2 * H_LOC + bh_ + 1) * T],
                    in_=qk_sb[:, a_, bh_, :])
        for bh_ in range(2 * H_LOC):
            nc.sync.dma_start(
                out=dumps["v_dump"][:, bh_ * (T // 128) * 132:
                                    (bh_ + 1) * (T // 128) * 132],
                in_=v_sb[:, bh_])
        for hl_ in range(H_LOC):
            nc.sync.dma_start(out=dumps["yT_dump"][:, hl_ * BT:(hl_ + 1) * BT],
                              in_=yT_sb[:, hl_])
        for hb_ in range(2):
            nc.sync.dma_start(
                out=dumps["ya_dump"][:, hb_ * N_CCH * 256:(hb_ + 1) * N_CCH * 256],
                in_=ya_sb[:, hb_])


def _schedule_phases(nc, tc, ctxA, L, phases):
    """Sequential schedule for phase-isolation timing builds."""
    x3_d, y_d = L["x3_d"], L["y_d"]
    wqk_d, wv_d = L["t_"]["wqk_d"], L["t_"]["wv_d"]
    cosT_d, sinTs_d = L["t_"]["cosT_d"], L["t_"]["sinTs_d"]
    wproj_d = L["t_"]["wproj_d"]
    a2a_in, a2a_out = L["a2a_in"], L["a2a_out"]
    use_collective = L["use_collective"]
    wproj_sb = L["wproj_sb"]
    qk_sb, v_sb, yT_sb, ya_sb = L["qk_sb"], L["v_sb"], L["yT_sb"], L["ya_sb"]
    ident, tri_sb = L["ident"], L["tri_sb"]
    qkv = L["qkv"]

    ptp = ctxA.enter_context(tc.tile_pool(name="ptp", bufs=3))
    osb = ctxA.enter_context(tc.tile_pool(name="osb", bufs=3))

    cq = contextlib.ExitStack()
    wp1 = cq.enter_context(tc.tile_pool(name="wp1", bufs=1))
    wqk_sb = wp1.tile([128, N_CCH, 512], BF16)
    wv_sb = wp1.tile([128, N_CCH, 256], BF16)
    cos_sb = wp1.tile([128, T], BF16)
    sin_sb = wp1.tile([128, T], BF16)
    for cc in range(N_CCH):
        nc.scalar.dma_start(out=wqk_sb[:, cc, :], in_=wqk_d[cc])
    nc.scalar.dma_start(out=wv_sb[:], in_=wv_d.transpose([1, 0, 2]))
    nc.gpsimd.dma_start(out=cos_sb[:], in_=cosT_d)
    nc.gpsimd.dma_start(out=sin_sb[:], in_=sinTs_d)
    for cc in range(N_CCH):
        nc.scalar.dma_start(out=wproj_sb[:, cc, :], in_=wproj_d[cc])
    xp = cq.enter_context(tc.tile_pool(name="xp", bufs=18))
    rp = cq.enter_context(tc.tile_pool(name="rp", bufs=2))
    tp = cq.enter_context(tc.tile_pool(name="tp", bufs=3))

    if 1 in phases:
        c1 = contextlib.ExitStack()
        with c1:
            P = {"xp": xp, "rp": rp, "tp": tp,
                 "qkps": c1.enter_context(
                     tc.tile_pool(name="qkps", bufs=2, space="PSUM")),
                 "vps": c1.enter_context(
                     tc.tile_pool(name="vps", bufs=2, space="PSUM"))}
            for b in range(B):
                _drain(_gen_qkv(nc, b, x3_d, wqk_sb, wv_sb, cos_sb, sin_sb,
                                qk_sb, v_sb, P))
    elif 2 in phases:
        ztmp = qkv.tile([128, 512], BF16)
        nc.vector.memset(ztmp[:], 0.001)
        nc.vector.memset(v_sb[:, :, :, 0:128], 0.001)
        for a_ in range(2):
            for bh_ in range(2 * H_LOC):
                for i_ in range(T // 512):
                    nc.vector.tensor_copy(
                        out=qk_sb[:, a_, bh_, i_ * 512:(i_ + 1) * 512],
                        in_=ztmp[:])
    if 3 in phases and 2 not in phases:
        ztmp2 = qkv.tile([128, 512], BF16)
        nc.vector.memset(ztmp2[:], 0.001)
        for hl_ in range(H_LOC):
            for i_ in range(BT // 512):
                nc.vector.tensor_copy(
                    out=yT_sb[:, hl_, i_ * 512:(i_ + 1) * 512], in_=ztmp2[:])

    for b in range(B):
        if 2 in phases:
            c2 = contextlib.ExitStack()
            with c2:
                P = {"ptp": ptp, "osb": osb,
                     "ops": c2.enter_context(
                         tc.tile_pool(name="ops", bufs=4, space="PSUM")),
                     "stps": c2.enter_context(
                         tc.tile_pool(name="stps", bufs=2, space="PSUM")),
                     "otps": c2.enter_context(
                         tc.tile_pool(name="otps", bufs=2, space="PSUM"))}
                _drain(_gen_attention(nc, b, qk_sb, v_sb, yT_sb, tri_sb,
                                      ident, P))
        if 3 in phases:
            _emit_spill_collective(nc, b, yT_sb, a2a_in, a2a_out, ya_sb,
                                   use_collective)
    cq.close()
    if 3 in phases:
        c3 = contextlib.ExitStack()
        with c3:
            PJ = {"outp": c3.enter_context(tc.tile_pool(name="outp", bufs=3)),
                  "pjps": c3.enter_context(
                      tc.tile_pool(name="pjps", bufs=6, space="PSUM"))}
            for hb in range(2):
                _drain(_gen_proj(nc, hb, ya_sb, wproj_sb, y_d, PJ))


# ---------------- host side ----------------

def _rope_tables():
    inv_freq = 1.0 / (ROPE_BASE ** (np.arange(0, D, 2, dtype=np.float32) / D))
    ang = np.arange(T, dtype=np.float32)[:, None] * inv_freq[None, :]   # (T, D/2)
    cos = np.concatenate([np.cos(ang), np.cos(ang)], axis=-1).astype(np.float32)
    sin = np.concatenate([np.sin(ang), np.sin(ang)], axis=-1).astype(np.float32)
    cosT = np.ascontiguousarray(cos.T)                                  # (D, T)
    sinTs = np.ascontiguousarray(sin.T)
    sinTs[0:64, :] *= -1.0
    return (cosT.astype(ml_dtypes.bfloat16), sinTs.astype(ml_dtypes.bfloat16))


def _tri_mask():
    kp = np.arange(128)[:, None]
    qf = np.arange(128)[None, :]
    return (kp <= qf).astype(ml_dtypes.bfloat16)                        # (128, 128)


def prep_in_maps(x, w_qkv, w_proj):
    bf = ml_dtypes.bfloat16
    x = np.asarray(x, dtype=np.float32)
    w_qkv = np.asarray(w_qkv, dtype=np.float32)
    w_proj = np.asarray(w_proj, dtype=np.float32)

    # (N_XB, C, 512): token-block-major transposed x
    x3 = np.ascontiguousarray(
        x.reshape(N_XB, 512, C).transpose(0, 2, 1)).astype(bf)
    wprojT = np.ascontiguousarray(w_proj.T).reshape(N_CCH, 128, C).astype(bf)
    cosT, sinTs = _rope_tables()
    tri = _tri_mask()

    in_maps = []
    for r in range(N_CORES):
        rows = slice(256 * r, 256 * (r + 1))
        wq = np.ascontiguousarray(w_qkv[0 * C:1 * C][rows].T).reshape(N_CCH, 128, 256)
        wk = np.ascontiguousarray(w_qkv[1 * C:2 * C][rows].T).reshape(N_CCH, 128, 256)
        wv = np.ascontiguousarray(w_qkv[2 * C:3 * C][rows].T).reshape(N_CCH, 128, 256)
        wqk = np.concatenate([wq, wk], axis=2)                           # (16,128,512)
        in_maps.append({
            "x3": x3, "wqk": np.ascontiguousarray(wqk).astype(bf),
            "wv": wv.astype(bf), "wproj": wprojT, "cosT": cosT,
            "sinTs": sinTs, "tri": tri,
        })
    return in_maps


def assemble(results):
    y0 = np.concatenate([results[r]["y"][0:256] for r in range(N_CORES)], axis=0)
    y1 = np.concatenate([results[r]["y"][256:512] for r in range(N_CORES)], axis=0)
    return np.stack([y0, y1], axis=0).reshape(B, T, C).astype(np.float32)


_CACHED_NC = None


def kernel(x, w_qkv, w_proj):
    global _CACHED_NC
    if _CACHED_NC is None:
        _CACHED_NC = build()
    in_maps = prep_in_maps(x, w_qkv, w_proj)
    res = run_bass_kernel_spmd(_CACHED_NC, in_maps, list(range(N_CORES)))
    return assemble(res.results)
